# revision 23
# baseline (speedup 1.0000x reference)
"""CapsuleLayer kernel for 8 trn2 NeuronCores.

Math (from the reference):
    c        = softmax(bias[0,:,:,0,0], axis=1)            # [I, J]
    s[b,j,d] = sum_{i,p} x[b,i,p] * W[i,j,p,d] * c[i,j]    # [B, J, D]
    out      = squash(s, axis=-1)

Folding c into W gives one big matmul
    s = X @ Wc,  X: [B, K], Wc: [K, N],  K = I*P = 32768, N = J*D = 1024.

Sharding: split the contraction dim K across the 8 cores (each core reads a
distinct 1/8 slice of W, so W is read exactly once fleet-wide — the memory
roofline optimum). Each core computes a partial [B, N] sum; the host adds
the 8 partials (2 MB total) and applies the tiny squash.

Precision/speed (MODE):
  "fp16"  — W cast to fp16 (pre-scaled by 2^8 so fp16(Wc) stays normal),
            x split into fp16 hi+lo (x = xh + xl). Products accumulate in
            the PE's fp32 PSUM, so the error is W's fp16 rounding,
            ~2^-12 ~ 2.4e-4 relative on the output. 2 full-rate passes per
            K-tile and half the DMA bytes of any fp32-exact scheme: the
            kernel sits at the DMA/PE ridge, ~27 us each per core.
  "bf16x3" — x and Wc split into bf16 hi+lo; s = xh@wh + xh@wl + xl@wh.
            bf16 products are exact in fp32 accumulation, so error is the
            dropped xl@wl term (~2^-18): measured 4.5e-6. Twice the DMA.

Layout: one input tensor per core, K-tile-major: each 128-row K-tile packs
[x parts | W parts] as contiguous columns, so a single chunked DMA stream
feeds everything (HWDGE FIFO completes chunks in order at full HBM
bandwidth; chunk sizes tapered small->large->small for pipeline latency).
Dummy matmuls on a memset tile pre-warm the PE's HAM clock gate during the
first chunk's DMA flight.
"""

import ml_dtypes
import numpy as np

import concourse.bass as bass
import concourse.mybir as mybir
import concourse.tile as tile
from concourse import bacc
from concourse.bass_utils import run_bass_kernel_spmd

MODE = "fp16x1"        # "fp16x1" | "fp16" | "bf16x3"

# Problem shapes (hardcoded per contract).
B, I, P, J, D = 64, 2048, 16, 32, 32
K = I * P            # 32768 contraction
N = J * D            # 1024 output features
N_CORES = 8
K_CORE = K // N_CORES  # 4096 contraction rows per core
KT = 128               # K-tile (partition dim of one matmul)
NKT = K_CORE // KT     # 32 K-tiles per core
# Tapered DMA chunk sizes (in K-tiles), summing to NKT. Small head chunks
# start the PE early; small tail chunks cut the last arrival->finish gap.
CHUNKS = [2, 3, 4, 4, 4, 4, 4, 3, 2, 2]
NB = N // 512          # PSUM-bank-sized slices of N (bank = 512 fp32)
N_WARM = 8             # dummy matmuls to lift the PE HAM clock gate
W_SCALE = 256.0        # exact power-of-2 lift keeping fp16(Wc) normal

if MODE == "fp16x1":
    NXP = 1            # x parts: single fp16
    NWP = 1            # w parts: single fp16
    NP_DTYPE = np.float16
    MM_DTYPE = mybir.dt.float16
    TERMS = [(0, 0)]
elif MODE == "fp16":
    NXP = 2            # x parts (hi, lo)
    NWP = 1
    NP_DTYPE = np.float16
    MM_DTYPE = mybir.dt.float16
    TERMS = [(0, 0), (1, 0)]
else:
    NXP = 2
    NWP = 2
    NP_DTYPE = ml_dtypes.bfloat16
    MM_DTYPE = mybir.dt.bfloat16
    TERMS = [(0, 0), (0, 1), (1, 0)]  # drops the lo@lo term

TC = NXP * B + NWP * N  # packed columns per K-tile

_NC_CACHE = None


def _build_nc():
    """Per-core program: out[B,N] accumulated over 32 K-tiles in PSUM."""
    nc = bacc.Bacc(trn_type="TRN2", target_bir_lowering=False, debug=False)
    f32 = mybir.dt.float32

    wx = nc.dram_tensor("wx", [KT, NKT * TC], MM_DTYPE, kind="ExternalInput")
    out = nc.dram_tensor("out", [B, N], f32, kind="ExternalOutput")

    assert sum(CHUNKS) == NKT
    n_small = sum(1 for s in CHUNKS if s <= 2)
    n_big = sum(1 for s in CHUNKS if s > 2)
    with tile.TileContext(nc) as tc:
        with (
            tc.tile_pool(name="cpool", bufs=1) as cpool,
            # One buffer per chunk (no slot reuse) so every chunk DMA can be
            # in flight at once; small/big pools so slots aren't all padded
            # to the largest chunk (SBUF budget).
            tc.tile_pool(name="wsmall", bufs=max(n_small, 1)) as wsmall,
            tc.tile_pool(name="wbig", bufs=max(n_big, 1)) as wbig,
            tc.tile_pool(name="opool", bufs=1) as opool,
            tc.tile_pool(name="pspool", bufs=1, space="PSUM") as pspool,
        ):
            # HAM warm-up: PE must stay busy ~3.4us to reach 2.4 GHz. These
            # dummies depend only on a memset tile, so they run during the
            # first chunk's DMA flight.
            warm = cpool.tile([KT, 512], MM_DTYPE)
            nc.vector.memset(warm[:], 1.0)
            warm_ps = pspool.tile([B, 512], f32)
            for _ in range(N_WARM):
                nc.tensor.matmul(
                    warm_ps[:], warm[:, 0:B], warm[:], start=True, stop=True
                )

            ps = pspool.tile([B, N], f32)

            def tile_views(w_sb, tl):
                base = tl * TC
                xp = [
                    w_sb[:, base + k * B : base + (k + 1) * B]
                    for k in range(NXP)
                ]
                wcol = [base + NXP * B + k * N for k in range(NWP)]
                return xp, wcol

            t = 0
            col = 0
            for ci, csz in enumerate(CHUNKS):
                pool = wsmall if csz <= 2 else wbig
                w_sb = pool.tile([KT, csz * TC], MM_DTYPE)
                nc.sync.dma_start(w_sb[:], wx.ap()[:, col : col + csz * TC])
                col += csz * TC
                if ci < len(CHUNKS) - 1:
                    # lhsT-major groups pair weight loads.
                    for tl in range(csz):
                        xp, wcol = tile_views(w_sb, tl)
                        for xi, wi in TERMS:
                            for nb in range(NB):
                                nc.tensor.matmul(
                                    ps[:, nb * 512 : (nb + 1) * 512],
                                    xp[xi],
                                    w_sb[:, wcol[wi] + nb * 512 : wcol[wi] + (nb + 1) * 512],
                                    start=(t + tl == 0 and (xi, wi) == TERMS[0]),
                                    stop=False,
                                )
                else:
                    # Last chunk goes bank-major so bank 0 finishes (stop)
                    # a whole chunk-half early and its PSUM eviction
                    # overlaps the bank 1 tail.
                    for nb in range(NB):
                        for tl in range(csz):
                            xp, wcol = tile_views(w_sb, tl)
                            for ti, (xi, wi) in enumerate(TERMS):
                                nc.tensor.matmul(
                                    ps[:, nb * 512 : (nb + 1) * 512],
                                    xp[xi],
                                    w_sb[:, wcol[wi] + nb * 512 : wcol[wi] + (nb + 1) * 512],
                                    start=False,
                                    stop=(tl == csz - 1 and ti == len(TERMS) - 1),
                                )
                t += csz

            # Per-bank eviction: bank 0's copy+store (and its HBM write
            # receipt) overlap bank 1's matmul tail and copy.
            o_sb = opool.tile([B, N], f32)
            nc.vector.tensor_copy(o_sb[:, 0:512], ps[:, 0:512])
            nc.sync.dma_start(out.ap()[:, 0:512], o_sb[:, 0:512])
            nc.vector.tensor_copy(o_sb[:, 512:1024], ps[:, 512:1024])
            nc.sync.dma_start(out.ap()[:, 512:1024], o_sb[:, 512:1024])
    # Run Bacc's compile pipeline (wait legalization, register allocation).
    # run_bass_via_pjrt serializes nc.m as-is and never finalizes.
    nc.finalize()
    return nc


def _get_nc():
    global _NC_CACHE
    if _NC_CACHE is None:
        _NC_CACHE = _build_nc()
    return _NC_CACHE


def _prepare_in_maps(inputs: np.ndarray, W: np.ndarray, bias: np.ndarray):
    """Fold softmax(bias) into W, split precision, pack K-tile-major."""
    x = np.asarray(inputs, dtype=np.float32)
    Wf = np.asarray(W, dtype=np.float32)
    b = np.asarray(bias, dtype=np.float32)[0, :, :, 0, 0]          # [I, J]

    # softmax over J per input capsule i (fp32, matches jax.nn.softmax).
    m = b.max(axis=1, keepdims=True)
    e = np.exp(b - m)
    c = e / e.sum(axis=1, keepdims=True)                            # [I, J]

    # Wc[(i,p),(j,d)] = W[i,j,p,d] * c[i,j]  ->  [K, N]
    wc = (Wf.transpose(0, 2, 1, 3) * c[:, None, :, None]).reshape(K, N)
    xT = np.ascontiguousarray(x.reshape(B, K).T)                    # [K, B]

    xh = xT.astype(NP_DTYPE)
    if NXP == 1:
        xparts = [xh]
    else:
        xl = (xT - xh.astype(np.float32)).astype(NP_DTYPE)
        xparts = [xh, xl]
    if NWP == 1:
        wparts = [(wc * np.float32(W_SCALE)).astype(NP_DTYPE)]
    else:
        wh = wc.astype(NP_DTYPE)
        wl = (wc - wh.astype(np.float32)).astype(NP_DTYPE)
        wparts = [wh, wl]

    packed = np.empty((K, TC), dtype=NP_DTYPE)
    for k in range(NXP):
        packed[:, k * B : (k + 1) * B] = xparts[k]
    for k in range(NWP):
        packed[:, NXP * B + k * N : NXP * B + (k + 1) * N] = wparts[k]

    in_maps = []
    for cid in range(N_CORES):
        sl = slice(cid * K_CORE, (cid + 1) * K_CORE)
        # K-tile-major packing: [NKT, KT, TC] -> [KT, NKT*TC]
        core = np.ascontiguousarray(
            packed[sl].reshape(NKT, KT, TC).swapaxes(0, 1).reshape(KT, NKT * TC)
        )
        in_maps.append({"wx": core})
    return in_maps


def _squash(s: np.ndarray) -> np.ndarray:
    s2 = np.sum(np.square(s), axis=-1, keepdims=True, dtype=np.float32)
    scale = s2 / (1.0 + s2) / np.sqrt(s2)
    return (scale * s).astype(np.float32)


def run(inputs, W, bias, **spmd_kwargs):
    """Full pipeline; returns (output, BassKernelResults)."""
    in_maps = _prepare_in_maps(inputs, W, bias)
    res = run_bass_kernel_spmd(
        _get_nc(), in_maps, core_ids=list(range(N_CORES)), **spmd_kwargs
    )
    s = np.zeros((B, N), dtype=np.float32)
    for r in res.results:
        s += np.asarray(r["out"], dtype=np.float32)
    if NWP == 1:
        s /= np.float32(W_SCALE)
    out = _squash(s.reshape(B, J, D))
    return out, res


def kernel(inputs, W, bias):
    out, _ = run(inputs, W, bias)
    return out


# revision 24
# speedup vs baseline: 1.0109x; 1.0109x over previous
"""CapsuleLayer kernel for 8 trn2 NeuronCores.

Math (from the reference):
    c        = softmax(bias[0,:,:,0,0], axis=1)            # [I, J]
    s[b,j,d] = sum_{i,p} x[b,i,p] * W[i,j,p,d] * c[i,j]    # [B, J, D]
    out      = squash(s, axis=-1)

Folding c into W gives one big matmul
    s = X @ Wc,  X: [B, K], Wc: [K, N],  K = I*P = 32768, N = J*D = 1024.

Sharding: split the contraction dim K across the 8 cores (each core reads a
distinct 1/8 slice of W, so W is read exactly once fleet-wide — the memory
roofline optimum). Each core computes a partial [B, N] sum; the host adds
the 8 partials (2 MB total) and applies the tiny squash.

Precision/speed (MODE):
  "fp16"  — W cast to fp16 (pre-scaled by 2^8 so fp16(Wc) stays normal),
            x split into fp16 hi+lo (x = xh + xl). Products accumulate in
            the PE's fp32 PSUM, so the error is W's fp16 rounding,
            ~2^-12 ~ 2.4e-4 relative on the output. 2 full-rate passes per
            K-tile and half the DMA bytes of any fp32-exact scheme: the
            kernel sits at the DMA/PE ridge, ~27 us each per core.
  "bf16x3" — x and Wc split into bf16 hi+lo; s = xh@wh + xh@wl + xl@wh.
            bf16 products are exact in fp32 accumulation, so error is the
            dropped xl@wl term (~2^-18): measured 4.5e-6. Twice the DMA.

Layout: one input tensor per core, K-tile-major: each 128-row K-tile packs
[x parts | W parts] as contiguous columns, so a single chunked DMA stream
feeds everything (HWDGE FIFO completes chunks in order at full HBM
bandwidth; chunk sizes tapered small->large->small for pipeline latency).
Dummy matmuls on a memset tile pre-warm the PE's HAM clock gate during the
first chunk's DMA flight.
"""

import ml_dtypes
import numpy as np

import concourse.bass as bass
import concourse.mybir as mybir
import concourse.tile as tile
from concourse import bacc
from concourse.bass_utils import run_bass_kernel_spmd

MODE = "fp16x1"        # "fp16x1" | "fp16" | "bf16x3"

# Problem shapes (hardcoded per contract).
B, I, P, J, D = 64, 2048, 16, 32, 32
K = I * P            # 32768 contraction
N = J * D            # 1024 output features
N_CORES = 8
K_CORE = K // N_CORES  # 4096 contraction rows per core
KT = 128               # K-tile (partition dim of one matmul)
NKT = K_CORE // KT     # 32 K-tiles per core
# Tapered DMA chunk sizes (in K-tiles), summing to NKT. Small head chunks
# start the PE early; small tail chunks cut the last arrival->finish gap.
CHUNKS = [2, 3, 4, 4, 4, 4, 4, 3, 2, 2]
NB = N // 512          # PSUM-bank-sized slices of N (bank = 512 fp32)
N_WARM = 8             # dummy matmuls to lift the PE HAM clock gate
W_SCALE = 256.0        # exact power-of-2 lift keeping fp16(Wc) normal

if MODE == "fp16x1":
    NXP = 1            # x parts: single fp16
    NWP = 1            # w parts: single fp16
    NP_DTYPE = np.float16
    MM_DTYPE = mybir.dt.float16
    TERMS = [(0, 0)]
elif MODE == "fp16":
    NXP = 2            # x parts (hi, lo)
    NWP = 1
    NP_DTYPE = np.float16
    MM_DTYPE = mybir.dt.float16
    TERMS = [(0, 0), (1, 0)]
else:
    NXP = 2
    NWP = 2
    NP_DTYPE = ml_dtypes.bfloat16
    MM_DTYPE = mybir.dt.bfloat16
    TERMS = [(0, 0), (0, 1), (1, 0)]  # drops the lo@lo term

TC = NXP * B + NWP * N  # packed columns per K-tile

_NC_CACHE = None


def _build_nc():
    """Per-core program: out[B,N] accumulated over 32 K-tiles in PSUM."""
    nc = bacc.Bacc(trn_type="TRN2", target_bir_lowering=False, debug=False)
    f32 = mybir.dt.float32

    wx = nc.dram_tensor("wx", [KT, NKT * TC], MM_DTYPE, kind="ExternalInput")
    out = nc.dram_tensor("out", [B, N], f32, kind="ExternalOutput")

    assert sum(CHUNKS) == NKT
    n_small = sum(1 for s in CHUNKS if s <= 2)
    n_big = sum(1 for s in CHUNKS if s > 2)
    with tile.TileContext(nc) as tc:
        with (
            tc.tile_pool(name="cpool", bufs=1) as cpool,
            # One buffer per chunk (no slot reuse) so every chunk DMA can be
            # in flight at once; small/big pools so slots aren't all padded
            # to the largest chunk (SBUF budget).
            tc.tile_pool(name="wsmall", bufs=max(n_small, 1)) as wsmall,
            tc.tile_pool(name="wbig", bufs=max(n_big, 1)) as wbig,
            tc.tile_pool(name="opool", bufs=1) as opool,
            tc.tile_pool(name="pspool", bufs=1, space="PSUM") as pspool,
        ):
            # HAM warm-up: PE must stay busy ~3.4us to reach 2.4 GHz. These
            # dummies depend only on a memset tile, so they run during the
            # first chunk's DMA flight.
            warm = cpool.tile([KT, 512], MM_DTYPE)
            nc.vector.memset(warm[:], 1.0)
            warm_ps = pspool.tile([B, 512], f32)
            for _ in range(N_WARM):
                nc.tensor.matmul(
                    warm_ps[:], warm[:, 0:B], warm[:], start=True, stop=True
                )

            ps = pspool.tile([B, N], f32)

            def tile_views(w_sb, tl):
                base = tl * TC
                xp = [
                    w_sb[:, base + k * B : base + (k + 1) * B]
                    for k in range(NXP)
                ]
                wcol = [base + NXP * B + k * N for k in range(NWP)]
                return xp, wcol

            t = 0
            col = 0
            for ci, csz in enumerate(CHUNKS):
                pool = wsmall if csz <= 2 else wbig
                w_sb = pool.tile([KT, csz * TC], MM_DTYPE)
                nc.sync.dma_start(w_sb[:], wx.ap()[:, col : col + csz * TC])
                col += csz * TC
                if ci < len(CHUNKS) - 1:
                    # lhsT-major groups pair weight loads.
                    for tl in range(csz):
                        xp, wcol = tile_views(w_sb, tl)
                        for xi, wi in TERMS:
                            for nb in range(NB):
                                nc.tensor.matmul(
                                    ps[:, nb * 512 : (nb + 1) * 512],
                                    xp[xi],
                                    w_sb[:, wcol[wi] + nb * 512 : wcol[wi] + (nb + 1) * 512],
                                    start=(t + tl == 0 and (xi, wi) == TERMS[0]),
                                    stop=False,
                                )
                else:
                    # Last chunk goes bank-major so bank 0 finishes (stop)
                    # a whole chunk-half early and its PSUM eviction
                    # overlaps the bank 1 tail.
                    for nb in range(NB):
                        for tl in range(csz):
                            xp, wcol = tile_views(w_sb, tl)
                            for ti, (xi, wi) in enumerate(TERMS):
                                nc.tensor.matmul(
                                    ps[:, nb * 512 : (nb + 1) * 512],
                                    xp[xi],
                                    w_sb[:, wcol[wi] + nb * 512 : wcol[wi] + (nb + 1) * 512],
                                    start=False,
                                    stop=(tl == csz - 1 and ti == len(TERMS) - 1),
                                )
                t += csz

            # Per-bank eviction: bank 0's copy+store (and its HBM write
            # receipt) overlap bank 1's matmul tail and copy.
            o_sb = opool.tile([B, N], f32)
            nc.vector.tensor_copy(o_sb[:, 0:512], ps[:, 0:512])
            nc.sync.dma_start(out.ap()[:, 0:512], o_sb[:, 0:512])
            nc.vector.tensor_copy(o_sb[:, 512:1024], ps[:, 512:1024])
            nc.sync.dma_start(out.ap()[:, 512:1024], o_sb[:, 512:1024])
    # Run Bacc's compile pipeline (wait legalization, register allocation).
    # run_bass_via_pjrt serializes nc.m as-is and never finalizes.
    nc.finalize()
    return nc


def _get_nc():
    global _NC_CACHE
    if _NC_CACHE is None:
        _NC_CACHE = _build_nc()
    return _NC_CACHE


def _prepare_in_maps(inputs: np.ndarray, W: np.ndarray, bias: np.ndarray):
    """Fold softmax(bias) into W, split precision, pack K-tile-major."""
    x = np.asarray(inputs, dtype=np.float32)
    Wf = np.asarray(W, dtype=np.float32)
    b = np.asarray(bias, dtype=np.float32)[0, :, :, 0, 0]          # [I, J]

    # softmax over J per input capsule i (fp32, matches jax.nn.softmax).
    m = b.max(axis=1, keepdims=True)
    e = np.exp(b - m)
    c = e / e.sum(axis=1, keepdims=True)                            # [I, J]

    # Wc[(i,p),(j,d)] = W[i,j,p,d] * c[i,j]  ->  [K, N]
    wc = (Wf.transpose(0, 2, 1, 3) * c[:, None, :, None]).reshape(K, N)
    xT = np.ascontiguousarray(x.reshape(B, K).T)                    # [K, B]

    xh = xT.astype(NP_DTYPE)
    if NXP == 1:
        xparts = [xh]
    else:
        xl = (xT - xh.astype(np.float32)).astype(NP_DTYPE)
        xparts = [xh, xl]
    if NWP == 1:
        wparts = [(wc * np.float32(W_SCALE)).astype(NP_DTYPE)]
    else:
        wh = wc.astype(NP_DTYPE)
        wl = (wc - wh.astype(np.float32)).astype(NP_DTYPE)
        wparts = [wh, wl]

    packed = np.empty((K, TC), dtype=NP_DTYPE)
    for k in range(NXP):
        packed[:, k * B : (k + 1) * B] = xparts[k]
    for k in range(NWP):
        packed[:, NXP * B + k * N : NXP * B + (k + 1) * N] = wparts[k]

    in_maps = []
    for cid in range(N_CORES):
        sl = slice(cid * K_CORE, (cid + 1) * K_CORE)
        # K-tile-major packing: [NKT, KT, TC] -> [KT, NKT*TC]
        core = np.ascontiguousarray(
            packed[sl].reshape(NKT, KT, TC).swapaxes(0, 1).reshape(KT, NKT * TC)
        )
        in_maps.append({"wx": core})
    return in_maps


def _squash(s: np.ndarray) -> np.ndarray:
    s2 = np.sum(np.square(s), axis=-1, keepdims=True, dtype=np.float32)
    scale = s2 / (1.0 + s2) / np.sqrt(s2)
    return (scale * s).astype(np.float32)


def run(inputs, W, bias, **spmd_kwargs):
    """Full pipeline; returns (output, BassKernelResults)."""
    in_maps = _prepare_in_maps(inputs, W, bias)
    try:
        res = run_bass_kernel_spmd(
            _get_nc(), in_maps, core_ids=list(range(N_CORES)), **spmd_kwargs
        )
    except Exception:
        # A crashed prior process can leave a core wedged
        # (NRT_EXEC_UNIT_UNRECOVERABLE); one retry clears it.
        import time
        time.sleep(2.0)
        res = run_bass_kernel_spmd(
            _get_nc(), in_maps, core_ids=list(range(N_CORES)), **spmd_kwargs
        )
    s = np.zeros((B, N), dtype=np.float32)
    for r in res.results:
        s += np.asarray(r["out"], dtype=np.float32)
    if NWP == 1:
        s /= np.float32(W_SCALE)
    out = _squash(s.reshape(B, J, D))
    return out, res


def kernel(inputs, W, bias):
    out, _ = run(inputs, W, bias)
    return out


# revision 26
# speedup vs baseline: 1.1138x; 1.1019x over previous
"""CapsuleLayer kernel for 8 trn2 NeuronCores.

Math (from the reference):
    c        = softmax(bias[0,:,:,0,0], axis=1)            # [I, J]
    s[b,j,d] = sum_{i,p} x[b,i,p] * W[i,j,p,d] * c[i,j]    # [B, J, D]
    out      = squash(s, axis=-1)

Folding c into W gives one big matmul
    s = X @ Wc,  X: [B, K], Wc: [K, N],  K = I*P = 32768, N = J*D = 1024.

Sharding: split the contraction dim K across the 8 cores (each core reads a
distinct 1/8 slice of W, so W is read exactly once fleet-wide — the memory
roofline optimum). Each core computes a partial [B, N] sum; the host adds
the 8 partials (2 MB total) and applies the tiny squash.

Precision/speed (MODE) — this problem family gates at rel_err < 2e-2:
  "fp16x1" — x and Wc cast to fp16 (Wc pre-scaled by 2^8 so it stays
            normal-range). Products accumulate in the PE's fp32 PSUM, so
            the error is the operands' fp16 rounding, measured 3.0e-4
            relative — 67x inside the gate. Minimum DMA bytes (9 MB/core)
            and a single full-rate pass: the kernel is DMA-bound at the
            ~25 us/core HBM roofline.
  "fp16"  — adds an x-lo pass (x = xh + xl): error 2.1e-4, same DMA.
  "bf16x3" — x and Wc split into bf16 hi+lo; s = xh@wh + xh@wl + xl@wh.
            bf16 products are exact in fp32 accumulation, so error is the
            dropped xl@wl term (~2^-18): measured 4.5e-6. Twice the DMA.

Layout: one input tensor per core, K-tile-major: each 128-row K-tile packs
[x parts | W parts] as contiguous columns, so a single chunked DMA stream
feeds everything (HWDGE FIFO completes chunks in order at full HBM
bandwidth; chunk sizes tapered small->large->small for pipeline latency).
Dummy matmuls on a memset tile pre-warm the PE's HAM clock gate during the
first chunk's DMA flight.
"""

import ml_dtypes
import numpy as np

import concourse.mybir as mybir
import concourse.tile as tile
from concourse import bacc
from concourse.bass_utils import run_bass_kernel_spmd

MODE = "fp16x1"        # "fp16x1" | "fp16" | "bf16x3"

# Problem shapes (hardcoded per contract).
B, I, P, J, D = 64, 2048, 16, 32, 32
K = I * P            # 32768 contraction
N = J * D            # 1024 output features
N_CORES = 8
K_CORE = K // N_CORES  # 4096 contraction rows per core
KT = 128               # K-tile (partition dim of one matmul)
NKT = K_CORE // KT     # 32 K-tiles per core
# Tapered DMA chunk sizes (in K-tiles), summing to NKT. Small head chunks
# start the PE early; small tail chunks cut the last arrival->finish gap.
CHUNKS = [2, 3, 4, 4, 4, 4, 4, 3, 2, 2]
NB = N // 512          # PSUM-bank-sized slices of N (bank = 512 fp32)
N_WARM = 8             # dummy matmuls to lift the PE HAM clock gate
W_SCALE = 256.0        # exact power-of-2 lift keeping fp16(Wc) normal

if MODE == "fp16x1":
    NXP = 1            # x parts: single fp16
    NWP = 1            # w parts: single fp16
    NP_DTYPE = np.float16
    MM_DTYPE = mybir.dt.float16
    TERMS = [(0, 0)]
elif MODE == "fp16":
    NXP = 2            # x parts (hi, lo)
    NWP = 1
    NP_DTYPE = np.float16
    MM_DTYPE = mybir.dt.float16
    TERMS = [(0, 0), (1, 0)]
else:
    NXP = 2
    NWP = 2
    NP_DTYPE = ml_dtypes.bfloat16
    MM_DTYPE = mybir.dt.bfloat16
    TERMS = [(0, 0), (0, 1), (1, 0)]  # drops the lo@lo term

TC = NXP * B + NWP * N  # packed columns per K-tile

_NC_CACHE = None


def _build_nc():
    """Per-core program: out[B,N] accumulated over 32 K-tiles in PSUM."""
    nc = bacc.Bacc(trn_type="TRN2", target_bir_lowering=False, debug=False)
    f32 = mybir.dt.float32

    wx = nc.dram_tensor("wx", [KT, NKT * TC], MM_DTYPE, kind="ExternalInput")
    out = nc.dram_tensor("out", [B, N], f32, kind="ExternalOutput")

    assert sum(CHUNKS) == NKT
    n_small = sum(1 for s in CHUNKS if s <= 2)
    n_big = sum(1 for s in CHUNKS if s > 2)
    with tile.TileContext(nc) as tc:
        with (
            tc.tile_pool(name="cpool", bufs=1) as cpool,
            # One buffer per chunk (no slot reuse) so every chunk DMA can be
            # in flight at once; small/big pools so slots aren't all padded
            # to the largest chunk (SBUF budget).
            tc.tile_pool(name="wsmall", bufs=max(n_small, 1)) as wsmall,
            tc.tile_pool(name="wbig", bufs=max(n_big, 1)) as wbig,
            tc.tile_pool(name="opool", bufs=1) as opool,
            tc.tile_pool(name="pspool", bufs=1, space="PSUM") as pspool,
        ):
            # HAM warm-up: PE must stay busy ~3.4us to reach 2.4 GHz. These
            # dummies depend only on a memset tile, so they run during the
            # first chunk's DMA flight.
            warm = cpool.tile([KT, 512], MM_DTYPE)
            nc.vector.memset(warm[:], 1.0)
            warm_ps = pspool.tile([B, 512], f32)
            for _ in range(N_WARM):
                nc.tensor.matmul(
                    warm_ps[:], warm[:, 0:B], warm[:], start=True, stop=True
                )

            ps = pspool.tile([B, N], f32)

            def tile_views(w_sb, tl):
                base = tl * TC
                xp = [
                    w_sb[:, base + k * B : base + (k + 1) * B]
                    for k in range(NXP)
                ]
                wcol = [base + NXP * B + k * N for k in range(NWP)]
                return xp, wcol

            t = 0
            col = 0
            for ci, csz in enumerate(CHUNKS):
                pool = wsmall if csz <= 2 else wbig
                w_sb = pool.tile([KT, csz * TC], MM_DTYPE)
                nc.sync.dma_start(w_sb[:], wx.ap()[:, col : col + csz * TC])
                col += csz * TC
                if ci < len(CHUNKS) - 1:
                    # lhsT-major groups pair weight loads.
                    for tl in range(csz):
                        xp, wcol = tile_views(w_sb, tl)
                        for xi, wi in TERMS:
                            for nb in range(NB):
                                nc.tensor.matmul(
                                    ps[:, nb * 512 : (nb + 1) * 512],
                                    xp[xi],
                                    w_sb[:, wcol[wi] + nb * 512 : wcol[wi] + (nb + 1) * 512],
                                    start=(t + tl == 0 and (xi, wi) == TERMS[0]),
                                    stop=False,
                                )
                else:
                    # Last chunk goes bank-major so bank 0 finishes (stop)
                    # a whole chunk-half early and its PSUM eviction
                    # overlaps the bank 1 tail.
                    for nb in range(NB):
                        for tl in range(csz):
                            xp, wcol = tile_views(w_sb, tl)
                            for ti, (xi, wi) in enumerate(TERMS):
                                nc.tensor.matmul(
                                    ps[:, nb * 512 : (nb + 1) * 512],
                                    xp[xi],
                                    w_sb[:, wcol[wi] + nb * 512 : wcol[wi] + (nb + 1) * 512],
                                    start=False,
                                    stop=(tl == csz - 1 and ti == len(TERMS) - 1),
                                )
                t += csz

            # Per-bank eviction: bank 0's copy+store (and its HBM write
            # receipt) overlap bank 1's matmul tail and copy.
            o_sb = opool.tile([B, N], f32)
            nc.vector.tensor_copy(o_sb[:, 0:512], ps[:, 0:512])
            nc.sync.dma_start(out.ap()[:, 0:512], o_sb[:, 0:512])
            nc.vector.tensor_copy(o_sb[:, 512:1024], ps[:, 512:1024])
            nc.sync.dma_start(out.ap()[:, 512:1024], o_sb[:, 512:1024])
    # Run Bacc's compile pipeline (wait legalization, register allocation).
    # run_bass_via_pjrt serializes nc.m as-is and never finalizes.
    nc.finalize()
    return nc


def _get_nc():
    global _NC_CACHE
    if _NC_CACHE is None:
        _NC_CACHE = _build_nc()
    return _NC_CACHE


def _prepare_in_maps(inputs: np.ndarray, W: np.ndarray, bias: np.ndarray):
    """Fold softmax(bias) into W, split precision, pack K-tile-major."""
    x = np.asarray(inputs, dtype=np.float32)
    Wf = np.asarray(W, dtype=np.float32)
    b = np.asarray(bias, dtype=np.float32)[0, :, :, 0, 0]          # [I, J]

    # softmax over J per input capsule i (fp32, matches jax.nn.softmax).
    m = b.max(axis=1, keepdims=True)
    e = np.exp(b - m)
    c = e / e.sum(axis=1, keepdims=True)                            # [I, J]

    # Wc[(i,p),(j,d)] = W[i,j,p,d] * c[i,j]  ->  [K, N]
    wc = (Wf.transpose(0, 2, 1, 3) * c[:, None, :, None]).reshape(K, N)
    xT = np.ascontiguousarray(x.reshape(B, K).T)                    # [K, B]

    xh = xT.astype(NP_DTYPE)
    if NXP == 1:
        xparts = [xh]
    else:
        xl = (xT - xh.astype(np.float32)).astype(NP_DTYPE)
        xparts = [xh, xl]
    if NWP == 1:
        wparts = [(wc * np.float32(W_SCALE)).astype(NP_DTYPE)]
    else:
        wh = wc.astype(NP_DTYPE)
        wl = (wc - wh.astype(np.float32)).astype(NP_DTYPE)
        wparts = [wh, wl]

    packed = np.empty((K, TC), dtype=NP_DTYPE)
    for k in range(NXP):
        packed[:, k * B : (k + 1) * B] = xparts[k]
    for k in range(NWP):
        packed[:, NXP * B + k * N : NXP * B + (k + 1) * N] = wparts[k]

    in_maps = []
    for cid in range(N_CORES):
        sl = slice(cid * K_CORE, (cid + 1) * K_CORE)
        # K-tile-major packing: [NKT, KT, TC] -> [KT, NKT*TC]
        core = np.ascontiguousarray(
            packed[sl].reshape(NKT, KT, TC).swapaxes(0, 1).reshape(KT, NKT * TC)
        )
        in_maps.append({"wx": core})
    return in_maps


def _squash(s: np.ndarray) -> np.ndarray:
    s2 = np.sum(np.square(s), axis=-1, keepdims=True, dtype=np.float32)
    scale = s2 / (1.0 + s2) / np.sqrt(s2)
    return (scale * s).astype(np.float32)


def run(inputs, W, bias, **spmd_kwargs):
    """Full pipeline; returns (output, BassKernelResults)."""
    in_maps = _prepare_in_maps(inputs, W, bias)
    try:
        res = run_bass_kernel_spmd(
            _get_nc(), in_maps, core_ids=list(range(N_CORES)), **spmd_kwargs
        )
    except Exception:
        # A crashed prior process can leave a core wedged
        # (NRT_EXEC_UNIT_UNRECOVERABLE); one retry clears it.
        import time
        time.sleep(2.0)
        res = run_bass_kernel_spmd(
            _get_nc(), in_maps, core_ids=list(range(N_CORES)), **spmd_kwargs
        )
    s = np.zeros((B, N), dtype=np.float32)
    for r in res.results:
        s += np.asarray(r["out"], dtype=np.float32)
    if NWP == 1:
        s /= np.float32(W_SCALE)
    out = _squash(s.reshape(B, J, D))
    return out, res


def kernel(inputs, W, bias):
    out, _ = run(inputs, W, bias)
    return out
